# revision 4
# baseline (speedup 1.0000x reference)
"""DegradationAttention TRN2 kernel.

Math (faithful to the reference):
    q, k are the *memory-reinterpreting* reshape of [B,L,H,E] -> [B*H, L, E]
    (mixes L and H exactly like torch .view on a contiguous tensor), v is the
    true per-head slice values[b, :, h, :].
    d2      = |q_l|^2 + |k_s|^2 - 2 q_l.k_s           (>= 0 mathematically)
    scores  = 1 - exp(-d2); causal mask; A = softmax(scores / 8)
    out     = A @ v

Implementation notes:
  * d2 is produced by ONE matmul via host-side augmentation:
        khat = [k, |k|^2, 1]  (66 x S),  qhat = [-2q, 1, |q|^2]  (66 x L)
    so khat^T @ qhat = d2^T (s on partitions, l free).  No max-subtraction is
    needed in the softmax: unmasked scaled scores are bounded in [0, 0.125]
    for ANY input, so exp never overflows.  The softmax numerator is
        exp(0.125*(1 - e1)) with e1 = exp(-d2)
    (the reference's max-subtraction only shifts by a row constant which
    cancels in the normalization).  Masked entries are zeroed by a 0/1
    upper-triangular multiply on the 16 diagonal 128x128 blocks only; all
    fully-masked blocks are never computed (causal tiling halves the work).
  * A ones-column appended to V makes the AV matmul also emit the softmax
    row-sums, so normalization is a [128,1] reciprocal + scale on the output.
  * batch*heads = 16 slices -> 2 per NeuronCore, no cross-core communication.
"""

import os
from contextlib import ExitStack

import ml_dtypes
import numpy as np

import concourse.bass as bass
import concourse.mybir as mybir
import concourse.tile as tile
from concourse import bacc
from concourse.bass_utils import run_bass_kernel_spmd
from concourse.masks import make_upper_triangular

B, L, S, H, E, D = 2, 2048, 2048, 8, 64, 64
N_CORES = 8
HPC = (B * H) // N_CORES  # head-slices per core = 2
NJ = S // 128  # s-chunks per head = 16
KAUG = E + 2  # 66
VW = D + 1  # 65: V plus ones column
SCALE = 0.125  # 1/sqrt(E)

# Column offset of A^T chunk j inside the per-head A buffer.  Chunk j holds
# columns l in [128*j, L) (the causally-reachable l for s-chunk j).
_A_OFFS = []
_o = 0
for _j in range(NJ):
    _A_OFFS.append(_o)
    _o += L - 128 * _j
A_COLS = _o  # 17408

TRACE = False  # test.py sets True to collect an NTFF profile
LAST = {}  # exec_time_ns etc. from the most recent run

_CACHE = {}


def _build_program():
    nc = bacc.Bacc(
        "TRN2", target_bir_lowering=False, debug=False, num_devices=N_CORES
    )
    bf16 = mybir.dt.bfloat16
    f32 = mybir.dt.float32
    AF = mybir.ActivationFunctionType

    qh_d = nc.dram_tensor("qhat", [HPC, KAUG, L], bf16, kind="ExternalInput").ap()
    kh_d = nc.dram_tensor("khat", [HPC, KAUG, S], bf16, kind="ExternalInput").ap()
    vh_d = nc.dram_tensor("vhat", [HPC, 128, NJ * VW], bf16, kind="ExternalInput").ap()
    out_d = nc.dram_tensor("out", [HPC, L, D], f32, kind="ExternalOutput").ap()

    with tile.TileContext(nc) as tc, ExitStack() as ctx:
        consts = ctx.enter_context(tc.tile_pool(name="consts", bufs=1))
        io = ctx.enter_context(tc.tile_pool(name="io", bufs=2))
        apool = ctx.enter_context(tc.tile_pool(name="apool", bufs=2))
        epool = ctx.enter_context(tc.tile_pool(name="epool", bufs=4))
        opool = ctx.enter_context(tc.tile_pool(name="opool", bufs=3))
        ps_s = ctx.enter_context(tc.tile_pool(name="ps_s", bufs=3, space="PSUM"))
        ps_o = ctx.enter_context(tc.tile_pool(name="ps_o", bufs=2, space="PSUM"))

        mask = consts.tile([128, 128], bf16, tag="mask")
        # mask[s, l] = 1 where l >= s (keep), else 0
        make_upper_triangular(nc, mask[:], val=1.0, diag=True)
        bias_scale = consts.tile([128, 1], f32, tag="bias_scale")
        nc.vector.memset(bias_scale[:], SCALE)

        for h in range(HPC):
            kh = io.tile([KAUG, S], bf16, tag="kh")
            nc.sync.dma_start(out=kh[:], in_=kh_d[h])
            qh = io.tile([KAUG, L], bf16, tag="qh")
            nc.sync.dma_start(out=qh[:], in_=qh_d[h])
            vh = io.tile([128, NJ * VW], bf16, tag="vh")
            nc.sync.dma_start(out=vh[:], in_=vh_d[h])
            A = apool.tile([128, A_COLS], bf16, tag="A")

            for t in range(NJ):
                s0 = 128 * t
                # ---- scores for s-chunk t over valid l in [s0, L) ----
                for l0 in range(s0, L, 512):
                    w = min(512, L - l0)
                    ps = ps_s.tile([128, 512], f32, tag="ps_s")
                    nc.tensor.matmul(
                        ps[:, :w],
                        kh[:, s0 : s0 + 128],
                        qh[:, l0 : l0 + w],
                        start=True,
                        stop=True,
                    )
                    e1 = epool.tile([128, 512], f32, tag="e1")
                    # e1 = exp(-d2)
                    nc.scalar.activation(e1[:, :w], ps[:, :w], AF.Exp, scale=-1.0)
                    # A = exp(0.125 - 0.125 * e1)
                    acol = _A_OFFS[t] + (l0 - s0)
                    nc.scalar.activation(
                        A[:, acol : acol + w],
                        e1[:, :w],
                        AF.Exp,
                        bias=bias_scale[:],
                        scale=-SCALE,
                    )
                # causal mask on the diagonal block
                nc.vector.tensor_mul(
                    A[:, _A_OFFS[t] : _A_OFFS[t] + 128],
                    A[:, _A_OFFS[t] : _A_OFFS[t] + 128],
                    mask[:],
                )

                # ---- AV for l-block t: accumulate over s-chunks j <= t ----
                po = ps_o.tile([128, VW], f32, tag="po")
                for j in range(t + 1):
                    acol = _A_OFFS[j] + 128 * (t - j)
                    nc.tensor.matmul(
                        po[:],
                        A[:, acol : acol + 128],
                        vh[:, VW * j : VW * (j + 1)],
                        start=(j == 0),
                        stop=(j == t),
                    )
                r = opool.tile([128, 1], f32, tag="r")
                nc.vector.reciprocal(r[:], po[:, D : D + 1])
                ot = opool.tile([128, D], f32, tag="ot")
                nc.vector.tensor_scalar_mul(ot[:], po[:, 0:D], r[:])
                nc.sync.dma_start(out=out_d[h, s0 : s0 + 128, :], in_=ot[:])

    nc.compile()
    return nc


def _prep_inputs(queries, keys, values):
    """Host-side augmentation; returns per-core input maps."""
    q = np.ascontiguousarray(np.asarray(queries, dtype=np.float32)).reshape(
        B * H, L, E
    )
    k = np.ascontiguousarray(np.asarray(keys, dtype=np.float32)).reshape(B * H, S, E)
    v = np.asarray(values, dtype=np.float32).transpose(0, 2, 1, 3).reshape(B * H, S, D)

    qq = np.einsum("nle,nle->nl", q, q)
    kk = np.einsum("nse,nse->ns", k, k)

    qhat = np.empty((B * H, KAUG, L), dtype=np.float32)
    qhat[:, :E, :] = -2.0 * q.transpose(0, 2, 1)
    qhat[:, E, :] = 1.0
    qhat[:, E + 1, :] = qq

    khat = np.empty((B * H, KAUG, S), dtype=np.float32)
    khat[:, :E, :] = k.transpose(0, 2, 1)
    khat[:, E, :] = kk
    khat[:, E + 1, :] = 1.0

    vhat = np.empty((B * H, S, VW), dtype=np.float32)
    vhat[:, :, :D] = v
    vhat[:, :, D] = 1.0
    # [n, S, VW] -> [n, 128, NJ*VW] with element (p, j*VW+d) = vhat[n, j*128+p, d]
    vhat = np.ascontiguousarray(
        vhat.reshape(B * H, NJ, 128, VW).transpose(0, 2, 1, 3).reshape(
            B * H, 128, NJ * VW
        )
    )

    bf = ml_dtypes.bfloat16
    qhat = qhat.astype(bf)
    khat = khat.astype(bf)
    vhat = vhat.astype(bf)

    in_maps = []
    for c in range(N_CORES):
        sl = slice(HPC * c, HPC * (c + 1))
        in_maps.append(
            {
                "qhat": np.ascontiguousarray(qhat[sl]),
                "khat": np.ascontiguousarray(khat[sl]),
                "vhat": np.ascontiguousarray(vhat[sl]),
            }
        )
    return in_maps


def kernel(queries, keys, values):
    if "nc" not in _CACHE:
        _CACHE["nc"] = _build_program()
    nc = _CACHE["nc"]

    in_maps = _prep_inputs(queries, keys, values)
    try:
        res = run_bass_kernel_spmd(
            nc,
            in_maps,
            core_ids=list(range(N_CORES)),
            trace=TRACE,
        )
    except ModuleNotFoundError:
        # NTFF profiling hook unavailable in this environment
        res = run_bass_kernel_spmd(
            nc, in_maps, core_ids=list(range(N_CORES)), trace=False
        )
    LAST["exec_time_ns"] = res.exec_time_ns
    LAST["mean_exec_time_ns"] = res.mean_exec_time_ns

    out = np.concatenate([r["out"] for r in res.results], axis=0)  # [B*H, L, D]
    out = out.reshape(B, H, L, D).transpose(0, 2, 1, 3)  # [B, L, H, D]
    return np.ascontiguousarray(out)
